# revision 1
# baseline (speedup 1.0000x reference)
"""Trainium2 Bass kernel for nn_CustomModel_7378753814834.

Computation (see reference):
    d2[b, d]  = sum_k (x[b, k, d] - w[k, d])^2          (B=128, K=49, D=2048)
    kv[s,b,d] = exp(-d2[b,d] / (2 sigma_s^2))           (S=5 sigmas)
    out[s*B + b, k, d] = kv[s, b, d]    for all k       -> (640, 49, 2048) f32

Sharding: split D across the 8 cores (DL = 256 per core). Each core gets
x[:, :, c*DL:(c+1)*DL] (contiguous per-core array), computes d2 for all 128
batches with batch on the SBUF partition axis, and writes its D-slice of the
full sigma-major output (640, 49, DL). Host concatenates along d.

This file must be self-contained (only numpy + concourse from the
environment); shapes are hardcoded.
"""

import numpy as np

import concourse.bass as bass
import concourse.tile as tile
from concourse import bacc, mybir
from concourse import bass_utils

B, K, D = 128, 49, 2048
NCORES = 8
DL = D // NCORES            # 256 d-columns per core
F = K * DL                  # 12544 free elements per partition
S = 5
SIGMAS = [1.0, 2.0, 3.0, 4.0, 5.0]
INVS = [1.0 / (2.0 * s * s) for s in SIGMAS]

FP32 = mybir.dt.float32

# Knobs for experimentation from test.py
TRACE = False
TRACE_DIR = None
LAST = None          # last BassKernelResults (exec_time_ns when TRACE)

_compiled = None     # cached compiled Bass module


def _build_kernel():
    nc = bacc.Bacc(
        "TRN2",
        target_bir_lowering=False,
        debug=False,
        enable_asserts=False,
        num_devices=NCORES,
    )
    x = nc.dram_tensor("x", [B, F], FP32, kind="ExternalInput")
    w = nc.dram_tensor("w", [B, F], FP32, kind="ExternalInput")
    out = nc.dram_tensor("out", [S * B, F], FP32, kind="ExternalOutput")

    from contextlib import ExitStack

    with tile.TileContext(nc) as tc, ExitStack() as ctx:
        pool = ctx.enter_context(tc.tile_pool(name="main", bufs=1))

        X = pool.tile([B, F], FP32)
        W = pool.tile([B, F], FP32)
        nc.sync.dma_start(X[:], x.ap())
        nc.sync.dma_start(W[:], w.ap())

        # diff = x - w ; sq = diff * diff (in place)
        DIFF = pool.tile([B, F], FP32)
        nc.vector.tensor_sub(DIFF[:], X[:], W[:])
        nc.vector.tensor_mul(DIFF[:], DIFF[:], DIFF[:])

        # d2[b, d] = sum_k sq[b, k, d]: view [128, K, DL] -> [128, DL, K],
        # reduce innermost axis.
        D2 = pool.tile([B, DL], FP32)
        sq_t = DIFF[:].rearrange("p (k d) -> p k d", k=K).transpose([0, 2, 1])
        nc.vector.tensor_reduce(
            out=D2[:], in_=sq_t, axis=mybir.AxisListType.X, op=mybir.AluOpType.add
        )

        # Per sigma: kv = exp(-inv * d2), then broadcast-write over k.
        out_v = out.ap().rearrange("(s b) f -> s b f", s=S)
        for s in range(S):
            KV = pool.tile([B, DL], FP32, name=f"kv{s}")
            nc.scalar.activation(
                KV[:], D2[:], mybir.ActivationFunctionType.Exp, scale=-INVS[s]
            )
            dst = out_v[s].rearrange("b (k d) -> b k d", k=K)
            src = KV[:].unsqueeze(1).broadcast_to([B, K, DL])
            nc.sync.dma_start(dst, src)

    nc.compile()
    return nc


def _get_compiled():
    global _compiled
    if _compiled is None:
        _compiled = _build_kernel()
    return _compiled


def kernel(x, weight):
    x = np.ascontiguousarray(np.asarray(x, dtype=np.float32))
    weight = np.ascontiguousarray(np.asarray(weight, dtype=np.float32))
    assert x.shape == (B, K, D) and weight.shape == (1, K, D)

    nc = _get_compiled()

    in_maps = []
    for c in range(NCORES):
        xs = np.ascontiguousarray(x[:, :, c * DL : (c + 1) * DL]).reshape(B, F)
        ws = np.ascontiguousarray(weight[0, :, c * DL : (c + 1) * DL]).reshape(1, F)
        wb = np.ascontiguousarray(np.broadcast_to(ws, (B, F)))
        in_maps.append({"x": xs, "w": wb})

    res = bass_utils.run_bass_kernel_spmd(
        nc,
        in_maps,
        core_ids=list(range(NCORES)),
        trace=TRACE,
        tmpdir=TRACE_DIR,
    )
    global LAST
    LAST = res

    out = np.empty((S * B, K, D), dtype=np.float32)
    for c in range(NCORES):
        out[:, :, c * DL : (c + 1) * DL] = res.results[c]["out"].reshape(S * B, K, DL)
    return out


# revision 2
# speedup vs baseline: 1.2688x; 1.2688x over previous
"""Trainium2 Bass kernel for nn_CustomModel_7378753814834.

Computation (see reference):
    d2[b, d]  = sum_k (x[b, k, d] - w[k, d])^2          (B=128, K=49, D=2048)
    kv[s,b,d] = exp(-d2[b,d] / (2 sigma_s^2))           (S=5 sigmas)
    out[s*B + b, k, d] = kv[s, b, d]    for all k       -> (640, 49, 2048) f32

Sharding: split D across the 8 cores (DL = 256 per core). Each core gets
x[:, :, c*DL:(c+1)*DL] transposed on host to d-major [b, d', k] (so the
k-reduction is over the contiguous innermost axis), computes d2 for all 128
batches with batch on the SBUF partition axis, and writes its D-slice of the
full sigma-major output (640, 49, DL). Host concatenates along d.

Pipeline per core:
  - w slice loaded once (50KB) into one partition; broadcast to all 128
    partitions on GpSimd in d'-chunks (no 6.4MB HBM broadcast read).
  - x loaded in NCH d'-chunks; per chunk: DVE sub -> ACT square -> DVE
    contiguous reduce over k into D2[:, chunk].
  - per sigma: ACT exp(-inv*d2), DVE-build KV7 (kv replicated 7x along k)
    and a single DMA per sigma writes kv broadcast over k (7 x 7KB
    descriptors per partition), alternating between the two HWDGE rings.
"""

import numpy as np

import concourse.bass as bass
import concourse.tile as tile
from concourse import bacc, mybir
from concourse import bass_utils

B, K, D = 128, 49, 2048
NCORES = 8
DL = D // NCORES            # 256 d-columns per core
F = K * DL                  # 12544 free elements per partition
S = 5
SIGMAS = [1.0, 2.0, 3.0, 4.0, 5.0]
INVS = [1.0 / (2.0 * s * s) for s in SIGMAS]
K7 = 7                      # 49 = 7 x 7

NCH = 8                     # d'-chunks for the load/compute pipeline
DC = DL // NCH              # 32 d'-columns per chunk

FP32 = mybir.dt.float32

# Knobs for experimentation from test.py
TRACE = False
TRACE_DIR = None
LAST = None          # last BassKernelResults (exec_time_ns when TRACE)

_compiled = None     # cached compiled Bass module


def _build_kernel():
    from contextlib import ExitStack

    nc = bacc.Bacc(
        "TRN2",
        target_bir_lowering=False,
        debug=False,
        enable_asserts=False,
        num_devices=NCORES,
    )
    # x is d-major: [b, d', k] flattened to [B, DL*K]
    x = nc.dram_tensor("x", [B, F], FP32, kind="ExternalInput")
    # w is d-major too: [d', k] flattened
    w = nc.dram_tensor("w", [1, F], FP32, kind="ExternalInput")
    # out keeps the graded layout: [(s b), k, d']
    out = nc.dram_tensor("out", [S * B, F], FP32, kind="ExternalOutput")

    with tile.TileContext(nc) as tc, ExitStack() as ctx:
        const = ctx.enter_context(tc.tile_pool(name="const", bufs=1))
        xin = ctx.enter_context(tc.tile_pool(name="xin", bufs=3))
        work = ctx.enter_context(tc.tile_pool(name="work", bufs=2))
        kvp = ctx.enter_context(tc.tile_pool(name="kvp", bufs=2))

        W1 = const.tile([1, F], FP32)
        nc.sync.dma_start(W1[:], w.ap())
        WB = const.tile([B, F], FP32)
        D2 = const.tile([B, DL], FP32)

        x_v = x.ap().rearrange("b (d k) -> b d k", k=K)
        for c in range(NCH):
            sl = slice(c * DC * K, (c + 1) * DC * K)
            # broadcast w chunk to all partitions (GpSimd; overlaps DMAs)
            nc.gpsimd.partition_broadcast(WB[:, sl], W1[0:1, sl], channels=B)

            Xc = xin.tile([B, DC * K], FP32, tag="xc")
            nc.sync.dma_start(Xc[:], x_v[:, c * DC : (c + 1) * DC, :])

            DIFF = work.tile([B, DC * K], FP32, tag="diff")
            nc.vector.tensor_sub(DIFF[:], Xc[:], WB[:, sl])
            SQ = work.tile([B, DC * K], FP32, tag="sq")
            nc.scalar.square(SQ[:], DIFF[:])
            nc.vector.tensor_reduce(
                out=D2[:, c * DC : (c + 1) * DC],
                in_=SQ[:].rearrange("b (d k) -> b d k", k=K),
                axis=mybir.AxisListType.X,
                op=mybir.AluOpType.add,
            )

        out_v = out.ap().rearrange("(s b) (ko f) -> s b ko f", s=S, ko=K7)
        for s in range(S):
            KV = kvp.tile([B, DL], FP32, tag="kv")
            nc.scalar.activation(
                KV[:], D2[:], mybir.ActivationFunctionType.Exp, scale=-INVS[s]
            )
            # replicate kv 7x along k (inner k7), f = [k7i, d'] = 7*256
            KV7 = kvp.tile([B, K7 * DL], FP32, tag="kv7")
            nc.vector.tensor_copy(
                KV7[:].rearrange("b (j d) -> b j d", j=K7),
                KV[:].unsqueeze(1).broadcast_to([B, K7, DL]),
            )
            # one DMA per sigma: outer k7 broadcast of the 7KB KV7 line
            src = KV7[:].unsqueeze(1).broadcast_to([B, K7, K7 * DL])
            eng = nc.sync if s % 2 == 0 else nc.scalar
            eng.dma_start(out_v[s], src)

    nc.compile()
    return nc


def _get_compiled():
    global _compiled
    if _compiled is None:
        _compiled = _build_kernel()
    return _compiled


def kernel(x, weight):
    x = np.asarray(x, dtype=np.float32)
    weight = np.asarray(weight, dtype=np.float32)
    assert x.shape == (B, K, D) and weight.shape == (1, K, D)

    nc = _get_compiled()

    in_maps = []
    for c in range(NCORES):
        # d-major per-core slices: [b, d', k]
        xs = np.ascontiguousarray(
            x[:, :, c * DL : (c + 1) * DL].transpose(0, 2, 1)
        ).reshape(B, F)
        ws = np.ascontiguousarray(
            weight[0, :, c * DL : (c + 1) * DL].T
        ).reshape(1, F)
        in_maps.append({"x": xs, "w": ws})

    res = bass_utils.run_bass_kernel_spmd(
        nc,
        in_maps,
        core_ids=list(range(NCORES)),
        trace=TRACE,
        tmpdir=TRACE_DIR,
    )
    global LAST
    LAST = res

    out = np.empty((S * B, K, D), dtype=np.float32)
    for c in range(NCORES):
        out[:, :, c * DL : (c + 1) * DL] = res.results[c]["out"].reshape(S * B, K, DL)
    return out
